# revision 1
# baseline (speedup 1.0000x reference)
"""Trainium2 Bass kernel for the YAT MixerBlock (nn_MixerBlock_12524124635797).

Strategy: pure data-parallel over batch (64 -> 8 per core). Each core runs
the full mixer block for its 8 batch elements.

Per-core dataflow (all GEMMs fp16 inputs, fp32 PSUM accumulation):
  Token stage (per batch b, x_b is (196p, 768c)):
    dot1 (384t-part, 768c-free) = twT.T @ x_b            [PE]
    den  = wn_t[t] + xn[c] - 2*dot1 + eps                [DVE affine_then_add]
    rec  = 1/den                                         [DVE reciprocal_approx_fast]
    sq   = (dot1 + tb[t])^2                              [ACT Square, bias slot]
    h1   = sq * rec  (fp16)                              [GPSIMD mult; scale_t folded into w2]
    x2T (768c-part, 196p-free) = h1.T@w2sT + x_b.T@I196 + ones.T@b2row   [PE, shortcut+bias
                                                          folded in as extra K rows]
  Channel stage (rows = (b,p) flattened, 1568 per core):
    xn2b (128, rows) = ones.T @ (x2T*x2T)                [PE broadcast of row norms]
    for row-block rb, for m-chunk mc (24 chunks of 3072):
      dot2 (128m-part, rows-free) = cwT.T @ x2T          [PE]
      den2/rec2/sq2/h2 as above (wn_c, cb per-partition) [DVE/ACT/GPSIMD]
      out_psum(rows-part, 768c) += h2.T @ w4sT[mc]       [PE]
    out_psum += x2T.T @ I768 + ones.T @ b4row            [PE, shortcut+bias]
    out (rows, 768) fp32 -> DRAM                         [ACT copy + DMA]
"""

import numpy as np

import concourse.bass as bass
import concourse.bacc as bacc
import concourse.mybir as mybir
from concourse import bass_utils
from concourse import tile

F16 = mybir.dt.float16
F32 = mybir.dt.float32
AF = mybir.ActivationFunctionType

EPS = 0.1
B, P, C, T, M3 = 64, 196, 768, 384, 3072
NCORES = 8
BL = B // NCORES          # 8 batches per core
ROWS = BL * P             # 1568 rows per core
ROWSP = 1664              # ROWS padded to a multiple of 128
RB = 256                  # row-block size for the channel stage (2 psum chunks)


def _ceil_div(a, b):
    return (a + b - 1) // b


def _n_slices(n, step=512):
    """Split [0, n) into matmul-legal free-dim slices (<=512, bank-aligned)."""
    out = []
    o = 0
    while o < n:
        out.append((o, min(step, n - o)))
        o += step
    return out


def build_program():
    nc = bacc.Bacc(
        "TRN2",
        target_bir_lowering=False,
        debug=False,
        enable_asserts=False,
        num_devices=NCORES,
    )

    # ---- DRAM I/O ----
    d = {}
    d["xa"] = nc.dram_tensor("xa", [BL, 128, C], F16, kind="ExternalInput").ap()
    d["xb"] = nc.dram_tensor("xb", [BL, 128, C], F16, kind="ExternalInput").ap()
    d["twT"] = nc.dram_tensor("twT", [128, 2, T], F16, kind="ExternalInput").ap()
    d["w2sT"] = nc.dram_tensor("w2sT", [128, 3, P], F16, kind="ExternalInput").ap()
    d["i196"] = nc.dram_tensor("i196", [128, 2, P], F16, kind="ExternalInput").ap()
    d["b2r"] = nc.dram_tensor("b2r", [1, P], F16, kind="ExternalInput").ap()
    d["cwT"] = nc.dram_tensor("cwT", [128, 6, M3], F16, kind="ExternalInput").ap()
    d["w4sT"] = nc.dram_tensor("w4sT", [128, 24, C], F16, kind="ExternalInput").ap()
    d["b4r"] = nc.dram_tensor("b4r", [1, C], F16, kind="ExternalInput").ap()
    d["wnt"] = nc.dram_tensor("wnt", [128, 3], F32, kind="ExternalInput").ap()
    d["tbc"] = nc.dram_tensor("tbc", [128, 3], F32, kind="ExternalInput").ap()
    d["wnc"] = nc.dram_tensor("wnc", [128, 24], F32, kind="ExternalInput").ap()
    d["cbc"] = nc.dram_tensor("cbc", [128, 24], F32, kind="ExternalInput").ap()
    out_dram = nc.dram_tensor("out", [ROWS, C], F32, kind="ExternalOutput").ap()

    with tile.TileContext(nc) as tc:
        with tc.tile_pool(name="consts", bufs=1) as cp:
            # Resident constants / persistent activations.
            twT = cp.tile([128, 2, T], F16)
            w2sT = cp.tile([128, 3, P], F16)
            i196 = cp.tile([128, 2, P], F16)
            b2r = cp.tile([128, P], F16)
            cwT = cp.tile([128, 6, M3], F16)
            w4sT = cp.tile([128, 24, C], F16)
            b4r = cp.tile([128, C], F16)
            wnt = cp.tile([128, 3], F32)
            tbc = cp.tile([128, 3], F32)
            wnc = cp.tile([128, 24], F32)
            cbc = cp.tile([128, 24], F32)
            ones = cp.tile([128, 128], F16)
            # Free dim padded to a multiple of 128 so the tail row-block's
            # 128-col DMA transpose reads stay in bounds (garbage cols unused).
            x2T = cp.tile([128, 6, ROWSP], F16)
            xn2b = cp.tile([128, ROWS], F32)

            # x input first (token stage's critical path) as two big strided
            # DMAs, then small token constants, all on the sync queue; the big
            # channel weights go on the scalar-engine HWDGE queue so they
            # don't block the token stage.
            # Per-batch x tiles: separate tiles so batch 0's consumers only
            # wait on batch 0's DMA. Startup-critical loads go first on sync;
            # big channel weights on the scalar queue.
            xbs = []
            nc.sync.dma_start(twT[:], d["twT"])
            for b in range(BL):
                xb = cp.tile([128, 2, C], F16, name=f"xb{b}")
                nc.sync.dma_start(xb[:, 0, :], d["xa"][b])
                nc.sync.dma_start(xb[0:68, 1, :], d["xb"][b, 0:68, :])
                xbs.append(xb)
                if b == 0:
                    nc.sync.dma_start(w2sT[:], d["w2sT"])
                    nc.sync.dma_start(i196[:], d["i196"])
                    nc.sync.dma_start(b2r[0:1, :], d["b2r"])
                    nc.sync.dma_start(wnt[:], d["wnt"])
                    nc.sync.dma_start(tbc[:], d["tbc"])
            nc.sync.dma_start(wnc[:], d["wnc"])
            nc.sync.dma_start(cbc[:], d["cbc"])
            nc.scalar.dma_start(cwT[:], d["cwT"])
            nc.scalar.dma_start(w4sT[:], d["w4sT"])
            nc.scalar.dma_start(b4r[0:1, :], d["b4r"])
            nc.vector.memset(ones[:], 1.0)
            nc.vector.memset(x2T[:, :, ROWS:ROWSP], 0.0)

            # ================= Token stage =================
            with (
                tc.tile_pool(name="tok_sbuf", bufs=2) as tp,
                tc.tile_pool(name="tok_psum", bufs=1, space="PSUM") as pp,
            ):
                for b in range(BL):
                    r0 = b * P
                    xb = xbs[b]

                    # dot1 first: it only needs twT + x, so the PE can start
                    # before the norm chain is ready.
                    dot1s = []
                    for tcn in range(3):
                        ps_dot1 = pp.tile(
                            [128, C], F32, tag="ps_dot1", bufs=2, name="ps_dot1"
                        )
                        for kc, kn in ((0, 128), (1, 68)):
                            for no, nn_ in _n_slices(C):
                                nc.tensor.matmul(
                                    ps_dot1[:, no : no + nn_],
                                    twT[0:kn, kc, tcn * 128 : (tcn + 1) * 128],
                                    xb[0:kn, kc, no : no + nn_],
                                    start=(kc == 0),
                                    stop=(kc == 1),
                                )
                        dot1s.append(ps_dot1)

                    # x-norm broadcast tile: xnb[q, c] = sum_p x[p, c]^2
                    xsq = tp.tile([128, 2, C], F16, tag="xsq")
                    nc.vector.tensor_mul(xsq[:, 0, :], xb[:, 0, :], xb[:, 0, :])
                    nc.vector.tensor_mul(
                        xsq[0:68, 1, :], xb[0:68, 1, :], xb[0:68, 1, :]
                    )
                    ps_xnb = pp.tile([128, C], F32, tag="ps_xnb", bufs=1)
                    for no, nn_ in _n_slices(C):
                        nc.tensor.matmul(
                            ps_xnb[:, no : no + nn_],
                            ones[:, :],
                            xsq[:, 0, no : no + nn_],
                            start=True,
                            stop=False,
                        )
                        nc.tensor.matmul(
                            ps_xnb[:, no : no + nn_],
                            ones[0:68, :],
                            xsq[0:68, 1, no : no + nn_],
                            start=False,
                            stop=True,
                        )
                    xnb = tp.tile([128, C], F32, tag="xnb")
                    nc.scalar.copy(xnb[:], ps_xnb[:])

                    h1 = tp.tile([128, 3, C], F16, tag="h1")
                    for tcn in range(3):
                        ps_dot1 = dot1s[tcn]
                        den = tp.tile([128, C], F32, tag="den")
                        nc.vector.affine_then_add(
                            den[:], ps_dot1[:], xnb[:],
                            scale=-2.0, bias=wnt[:, tcn : tcn + 1],
                        )
                        rec = tp.tile([128, C], F32, tag="rec")
                        nc.vector.reciprocal_approx_fast(rec[:], den[:])
                        sq = tp.tile([128, C], F32, tag="sq")
                        nc.scalar.activation(
                            sq[:], ps_dot1[:], AF.Square, bias=tbc[:, tcn : tcn + 1]
                        )
                        nc.gpsimd.tensor_mul(h1[:, tcn, :], sq[:], rec[:])

                    # token linear + shortcut + bias -> x2T columns for batch b
                    for mc in range(6):
                        ps_x2 = pp.tile([128, P], F32, tag="ps_x2", bufs=2)
                        for kc in range(3):
                            nc.tensor.matmul(
                                ps_x2[:],
                                h1[:, kc, mc * 128 : (mc + 1) * 128],
                                w2sT[:, kc, :],
                                start=(kc == 0),
                                stop=False,
                            )
                        for kc, kn in ((0, 128), (1, 68)):
                            nc.tensor.matmul(
                                ps_x2[:],
                                xb[0:kn, kc, mc * 128 : (mc + 1) * 128],
                                i196[0:kn, kc, :],
                                start=False,
                                stop=False,
                            )
                        nc.tensor.matmul(
                            ps_x2[:],
                            ones[0:1, :],
                            b2r[0:1, :],
                            start=False,
                            stop=True,
                        )
                        nc.scalar.copy(x2T[:, mc, r0 : r0 + P], ps_x2[:])

            # ================= Channel-stage row norms =================
            with (
                tc.tile_pool(name="xn_sbuf", bufs=1) as xp,
                tc.tile_pool(name="xn_psum", bufs=1, space="PSUM") as xpp,
            ):
                ps_xn2 = xpp.tile([128, ROWS], F32)
                for kc in range(6):
                    x2sq = xp.tile([128, ROWS], F16, tag="x2sq", bufs=2)
                    nc.vector.tensor_mul(x2sq[:], x2T[:, kc, 0:ROWS], x2T[:, kc, 0:ROWS])
                    for no, nn_ in _n_slices(ROWS):
                        nc.tensor.matmul(
                            ps_xn2[:, no : no + nn_],
                            ones[:, :],
                            x2sq[:, no : no + nn_],
                            start=(kc == 0),
                            stop=(kc == 5),
                        )
                nc.scalar.copy(xn2b[:], ps_xn2[:])

            # ================= Channel stage =================
            with (
                tc.tile_pool(name="ch_sbuf", bufs=2) as chp,
                tc.tile_pool(name="ch_psum", bufs=1, space="PSUM") as cpp,
            ):
                for r0 in range(0, ROWS, RB):
                    rn = min(RB, ROWS - r0)
                    nsub = _ceil_div(rn, 128)
                    po = [
                        cpp.tile([128, C], F32, tag=f"po{s}", bufs=1, name=f"po{s}")
                        for s in range(nsub)
                    ]
                    for mc in range(24):
                        ps_d2 = cpp.tile([128, RB], F32, tag="ps_d2", bufs=4)
                        for kc in range(6):
                            nc.tensor.matmul(
                                ps_d2[:, 0:rn],
                                cwT[:, kc, mc * 128 : (mc + 1) * 128],
                                x2T[:, kc, r0 : r0 + rn],
                                start=(kc == 0),
                                stop=(kc == 5),
                            )
                        den2 = chp.tile([128, RB], F32, tag="den2", bufs=4)
                        nc.vector.affine_then_add(
                            den2[:, 0:rn], ps_d2[:, 0:rn], xn2b[:, r0 : r0 + rn],
                            scale=-2.0, bias=wnc[:, mc : mc + 1],
                        )
                        rec2 = chp.tile([128, RB], F32, tag="rec2", bufs=4)
                        nc.vector.reciprocal_approx_fast(rec2[:, 0:rn], den2[:, 0:rn])
                        sq2 = chp.tile([128, RB], F32, tag="sq2", bufs=4)
                        nc.scalar.activation(
                            sq2[:, 0:rn], ps_d2[:, 0:rn], AF.Square,
                            bias=cbc[:, mc : mc + 1],
                        )
                        h2 = chp.tile([128, RB], F16, tag="h2", bufs=4)
                        # Alternate the multiply between GPSIMD and DVE so the
                        # last link of the yat chain isn't serialized on one
                        # engine's FIFO.
                        mul_eng = nc.gpsimd if mc % 3 else nc.vector
                        mul_eng.tensor_mul(h2[:, 0:rn], sq2[:, 0:rn], rec2[:, 0:rn])

                        for s in range(nsub):
                            sn = min(128, rn - s * 128)
                            for no, nn_ in _n_slices(C):
                                nc.tensor.matmul(
                                    po[s][0:sn, no : no + nn_],
                                    h2[:, s * 128 : s * 128 + sn],
                                    w4sT[:, mc, no : no + nn_],
                                    start=(mc == 0),
                                    stop=False,
                                )
                    # bias b4 row, then shortcut x2 added via DVE from a
                    # DMA-transposed copy of x2T (cheaper than routing the
                    # identity through the PE).
                    for s in range(nsub):
                        sn = min(128, rn - s * 128)
                        rs = r0 + s * 128
                        for no, nn_ in _n_slices(C):
                            nc.tensor.matmul(
                                po[s][0:sn, no : no + nn_],
                                ones[0:1, 0:sn],
                                b4r[0:1, no : no + nn_],
                                start=False,
                                stop=True,
                            )
                        x2row = chp.tile([128, 6, 128], F16, tag="x2row", bufs=3)
                        for kc in range(6):
                            # Always a full 128-col source block (x2T free dim
                            # is padded); extra rows of x2row are unused.
                            nc.sync.dma_start_transpose(
                                x2row[:, kc, :], x2T[:, kc, rs : rs + 128]
                            )
                        osb = chp.tile([128, C], F32, tag="osb", bufs=3)
                        nc.vector.tensor_add(
                            osb[0:sn, :],
                            po[s][0:sn, :],
                            x2row[0:sn, :, :].rearrange("p a b -> p (a b)"),
                        )
                        nc.sync.dma_start(out_dram[rs : rs + sn, :], osb[0:sn, :])

    nc.compile()
    return nc


def _pack_kpn(w, n_chunks):
    """(K, N) fp32 -> (128, n_chunks, N) fp16 with zero padding of K."""
    k, n = w.shape
    out = np.zeros((n_chunks * 128, n), np.float16)
    out[:k] = w.astype(np.float16)
    return np.ascontiguousarray(
        out.reshape(n_chunks, 128, n).transpose(1, 0, 2)
    )


def _pack_col(v, n_chunks):
    """(K,) fp32 -> (128, n_chunks) fp32 column chunks."""
    out = np.zeros((n_chunks * 128,), np.float32)
    out[: v.shape[0]] = v.astype(np.float32)
    return np.ascontiguousarray(out.reshape(n_chunks, 128).T)


_PROGRAM = None


def _get_program():
    global _PROGRAM
    if _PROGRAM is None:
        _PROGRAM = build_program()
    return _PROGRAM


def kernel(x, tw, tb, t_alpha, w2, b2, cw, cb, c_alpha, w4, b4, _trace=False):
    x = np.asarray(x, np.float32)
    tw = np.asarray(tw, np.float32)
    tb = np.asarray(tb, np.float32)
    w2 = np.asarray(w2, np.float32)
    b2 = np.asarray(b2, np.float32)
    cw = np.asarray(cw, np.float32)
    cb = np.asarray(cb, np.float32)
    w4 = np.asarray(w4, np.float32)
    b4 = np.asarray(b4, np.float32)

    # YAT output scales (exactly as the reference computes them), folded into
    # the following linear layers' weights and biases' stays separate.
    scale_t = np.float32(np.sqrt(np.float32(T / np.log(T + 1.0)))) ** np.asarray(
        t_alpha, np.float32
    )[0]
    scale_c = np.float32(np.sqrt(np.float32(M3 / np.log(M3 + 1.0)))) ** np.asarray(
        c_alpha, np.float32
    )[0]
    w2s = (w2 * scale_t).astype(np.float32)   # (P, T)
    w4s = (w4 * scale_c).astype(np.float32)   # (C, M3)

    shared = {
        "twT": _pack_kpn(tw.T, 2),                       # (196,384) -> (128,2,384)
        "w2sT": _pack_kpn(w2s.T, 3),                     # (384,196) -> (128,3,196)
        "i196": _pack_kpn(np.eye(P, dtype=np.float32), 2),
        "b2r": b2.astype(np.float16).reshape(1, P),
        "cwT": _pack_kpn(cw.T, 6),                       # (768,3072)
        "w4sT": _pack_kpn(w4s.T, 24),                    # (3072,768)
        "b4r": b4.astype(np.float16).reshape(1, C),
        "wnt": _pack_col((tw.astype(np.float32) ** 2).sum(1) + EPS, 3),
        "tbc": _pack_col(tb, 3),
        "wnc": _pack_col((cw.astype(np.float32) ** 2).sum(1) + EPS, 24),
        "cbc": _pack_col(cb, 24),
    }
    x16 = x.astype(np.float16).reshape(NCORES, BL, P, C)
    xa = np.ascontiguousarray(x16[:, :, 0:128, :])
    xbp = np.zeros((NCORES, BL, 128, C), np.float16)
    xbp[:, :, 0:68] = x16[:, :, 128:P, :]
    in_maps = [dict(shared, xa=xa[c], xb=xbp[c]) for c in range(NCORES)]

    nc = _get_program()
    kwargs = {}
    if _trace:
        import shutil

        shutil.rmtree("/tmp/bass_ntff", ignore_errors=True)
        import os

        os.makedirs("/tmp/bass_ntff", exist_ok=True)
        kwargs["tmpdir"] = "/tmp/bass_ntff"
    res = bass_utils.run_bass_kernel_spmd(
        nc, in_maps, core_ids=list(range(NCORES)), trace=_trace, **kwargs
    )
    out = np.concatenate([res.results[c]["out"] for c in range(NCORES)], axis=0)
    out = out.reshape(B, P, C).astype(np.float32)
    if _trace:
        kernel.last_results = res
    return out



# revision 5
# speedup vs baseline: 1.5291x; 1.5291x over previous
"""Trainium2 Bass kernel for the YAT MixerBlock (nn_MixerBlock_12524124635797).

Data-parallel over batch (64 -> 8 per core); fp8e4 DoubleRow matmuls for all
four GEMMs; fused custom-DVE epilogue.

Numerics (per core, all PSUM fp32):
  Token (per batch b, xt is (C=768 rows, P=196)):
    ps_dot1[t, c] = -2*dot  (twT8 = fp8(-2*tw), x8 = fp8(x), DoubleRow K=256)
    rec = DEN_RECIP(ps_dot1, xnb, W't) = 1/(wn+eps+2tb + xn - 2dot)   [DVE]
    sq  = (32*dot + 32*tb)^2 = 1024*(dot+tb)^2                        [ACT]
    h1_8 = sq*rec = (1024/scale_t)*yat_t   (fp8)                      [DVE/GPSIMD]
    ps_x2[c, p] = h1_8 @ w2s8T + 1024*x.T + 1024*b2   (w2s8 = fp8(scale_t*w2))
    x2T = ACT copy ps_x2/1024 (fp16); x2T8 = copy (fp8)
  Channel (rows r = 8*196 = 1568, padded 1664; blocks of 416):
    ps_d2[m, r] = -2*dot   (cw8 = fp8(-2*cw), x2T8, DoubleRow)
    rec2 = DEN_RECIP(ps_d2, xn2b, W'c); sq2 = 1024*(dot+cb)^2
    h2_8 = sq2*rec2 = (1024/scale_c)*yat_c  (fp8)
    poT[c, r] += w4T8 @ h2_8  (transposed out; w4s8 = fp8(scale_c*w4))
    out = affine_then_add(poT/1024 + b4) + x2T  -> fp16 -> DRAM (transposed)
"""

import numpy as np
import ml_dtypes

import concourse.bass as bass
import concourse.bacc as bacc
import concourse.mybir as mybir
from concourse import bass_utils
from concourse import tile

F8 = mybir.dt.float8e4
F16 = mybir.dt.float16
F32 = mybir.dt.float32
AF = mybir.ActivationFunctionType
DR = mybir.MatmulPerfMode.DoubleRow
NP_F8 = ml_dtypes.float8_e4m3

EPS = 0.1
B, P, C, T, M3 = 64, 196, 768, 384, 3072
NCORES = 8
BL = B // NCORES          # 8 batches per core
ROWS = BL * P             # 1568 rows per core
ROWSP = 1664              # padded to 4*416
RB = 416                  # channel row-block (4 uniform blocks of ROWSP)
RC0, RC1 = -0.23549792, 2.0017324   # 1-NR bitwise reciprocal consts

# ---------------------------------------------------------------------------
# Custom fused DVE ops (registered into concourse.dve_ops at import time).
# DEN_RECIP: out = recip1((in0 + s1) + in1)  -- den build + ~0.17%-accurate
#            bitwise-NOT seeded reciprocal with one Newton pass (7/8 stages).
# SQ_MUL:    out = (in0*s0 + s1)^2 * in1     -- fused square+scale+mult.
# ---------------------------------------------------------------------------
import concourse.dve_ops as dve_ops
from concourse.dve_spec import C0 as _C0, C1 as _C1, C2 as _C2
from concourse.dve_spec import AluOp as _AluOp, Bin as _Bin, Spec as _Spec
from concourse.dve_spec import Src0 as _S0, Src1 as _S1, lower as _dve_lower
from concourse.dve_uop import DveOpSpec as _DveOpSpec


def _make_op(name, spec):
    if any(op.name == name for op in dve_ops.OPS):
        return next(op for op in dve_ops.OPS if op.name == name)
    row = dve_ops._CUSTOM_DVE_ROW_BASE + len(dve_ops.OPS)
    shas = {}
    for ver in ("v3", "v4"):
        try:
            uops = _dve_lower(spec, ver=ver)
            shas[ver] = _DveOpSpec(
                name=name, opcode=row, uops=uops, rd1_en=True
            ).sha(ver)
        except Exception:
            pass
    op = dve_ops.DveOp(name=name, spec=spec, subdim=False, uops_sha=shas)
    dve_ops.OPS.append(op)
    dve_ops._SUB_OPCODE_FOR_NAME[name] = row
    dve_ops.CUSTOM_DVE_SPECS[name] = spec
    return op


def _ref_den_recip(in0, in1, s0, s1, imm2):
    den = (in0.astype(np.float32) + s1) + in1.astype(np.float32)
    nx = (~den.view(np.int32)).view(np.float32)
    y0 = nx * np.float32(s0)
    return y0 * (np.float32(imm2) - den * y0)


_den = (_S0 + _C1) + _S1
_nx = _Bin(_AluOp.BITWISE_NOT, _den, _den)
_y0 = _nx * _C0
DEN_RECIP_ANT = _make_op(
    "DEN_RECIP_ANT",
    _Spec(body=_y0 * (_C2 - _den * _y0), reference=_ref_den_recip),
)

_u = _S0 * _C0 + _C1
SQ_MUL_ANT = _make_op(
    "SQ_MUL_ANT",
    _Spec(
        body=(_u * _u) * _S1,
        reference=lambda in0, in1, s0, s1, imm2: (
            (in0.astype(np.float32) * s0 + s1) ** 2 * in1.astype(np.float32)
        ),
    ),
)


def _ceil_div(a, b):
    return (a + b - 1) // b


def _n_slices(n, step=512):
    out = []
    o = 0
    while o < n:
        out.append((o, min(step, n - o)))
        o += step
    return out


def build_program():
    nc = bacc.Bacc(
        "TRN2",
        target_bir_lowering=False,
        debug=False,
        enable_asserts=False,
        num_devices=NCORES,
    )

    # ---- DRAM I/O ----
    d = {}
    d["x16"] = nc.dram_tensor("x16", [BL, 128, 2, C], F16, kind="ExternalInput").ap()
    d["x8"] = nc.dram_tensor("x8", [BL, 128, 2, C], F8, kind="ExternalInput").ap()
    d["twT8"] = nc.dram_tensor("twT8", [128, 2, T], F8, kind="ExternalInput").ap()
    d["w2s8T"] = nc.dram_tensor("w2s8T", [128, 3, P], F8, kind="ExternalInput").ap()
    d["i196"] = nc.dram_tensor("i196", [128, 2, P], F16, kind="ExternalInput").ap()
    d["b2r"] = nc.dram_tensor("b2r", [1, P], F16, kind="ExternalInput").ap()
    d["cw8"] = nc.dram_tensor("cw8", [128, 6, M3], F8, kind="ExternalInput").ap()
    d["w4T8"] = nc.dram_tensor("w4T8", [128, 24, C], F8, kind="ExternalInput").ap()
    d["sqbt"] = nc.dram_tensor("sqbt", [128, 3], F32, kind="ExternalInput").ap()
    d["wpt"] = nc.dram_tensor("wpt", [128, 3], F32, kind="ExternalInput").ap()
    d["sqbc"] = nc.dram_tensor("sqbc", [128, 24], F32, kind="ExternalInput").ap()
    d["wpc"] = nc.dram_tensor("wpc", [128, 24], F32, kind="ExternalInput").ap()
    d["b4c"] = nc.dram_tensor("b4c", [128, 6], F32, kind="ExternalInput").ap()
    out_dram = nc.dram_tensor("outT", [128, 6, ROWS], F16, kind="ExternalOutput").ap()

    with tile.TileContext(nc) as tc:
        with tc.tile_pool(name="consts", bufs=1) as cp:
            twT8 = cp.tile([128, 2, T], F8)
            w2s8T = cp.tile([128, 3, P], F8)
            i196 = cp.tile([128, 2, P], F16)
            b2r = cp.tile([128, P], F16)
            cw8 = cp.tile([128, 6, M3], F8)
            w4T8 = cp.tile([128, 24, C], F8)
            sqbt = cp.tile([128, 3], F32)
            wpt = cp.tile([128, 3], F32)
            sqbc = cp.tile([128, 24], F32)
            wpc = cp.tile([128, 24], F32)
            b4c = cp.tile([128, 6], F32)
            ones = cp.tile([128, 128], F16)
            x2T = cp.tile([128, 6, ROWSP], F16)
            x2T8 = cp.tile([128, 6, ROWSP], F8)
            xn2b = cp.tile([128, ROWSP], F16)

            # startup-critical loads on the sync queue, channel weights on
            # the scalar-engine queue.
            nc.sync.dma_start(twT8[:], d["twT8"])
            xb16s, xb8s = [], []
            for b in range(BL):
                xb16 = cp.tile([128, 2, C], F16, name=f"xb16_{b}")
                xb8 = cp.tile([128, 2, C], F8, name=f"xb8_{b}")
                nc.sync.dma_start(xb16[:], d["x16"][b])
                nc.sync.dma_start(xb8[:], d["x8"][b])
                xb16s.append(xb16)
                xb8s.append(xb8)
                if b == 0:
                    nc.sync.dma_start(w2s8T[:], d["w2s8T"])
                    nc.sync.dma_start(i196[:], d["i196"])
                    nc.sync.dma_start(b2r[0:1, :], d["b2r"])
                    nc.sync.dma_start(sqbt[:], d["sqbt"])
                    nc.sync.dma_start(wpt[:], d["wpt"])
            nc.scalar.dma_start(cw8[:], d["cw8"])
            nc.scalar.dma_start(w4T8[:], d["w4T8"])
            nc.scalar.dma_start(sqbc[:], d["sqbc"])
            nc.scalar.dma_start(wpc[:], d["wpc"])
            nc.scalar.dma_start(b4c[:], d["b4c"])
            nc.vector.memset(ones[:], 1.0)
            nc.vector.memset(x2T[:, :, ROWS:ROWSP], 0.0)
            nc.vector.memset(x2T8[:, :, ROWS:ROWSP], 0.0)

            # ================= Token stage =================
            # Software-pipelined: yat-stage(b) then x2-stage(b-1), so the PE's
            # x2 matmuls wait a full batch behind the epilogue that feeds them.
            with (
                tc.tile_pool(name="tok_sbuf", bufs=2) as tp,
                tc.tile_pool(name="tok_psum", bufs=1, space="PSUM") as pp,
            ):
                h1s = [None] * BL

                def yat_stage(b):
                    xb16, xb8 = xb16s[b], xb8s[b]
                    dot1s = []
                    for tcn in range(3):
                        ps_dot1 = pp.tile(
                            [128, C], F32, tag="ps_dot1", bufs=2, name="ps_dot1"
                        )
                        for no, nn_ in _n_slices(C):
                            nc.tensor.matmul(
                                ps_dot1[:, no : no + nn_],
                                twT8[:, :, tcn * 128 : (tcn + 1) * 128],
                                xb8[:, :, no : no + nn_],
                                start=True,
                                stop=True,
                                perf_mode=DR,
                            )
                        dot1s.append(ps_dot1)

                    xsq = tp.tile([128, 2, C], F16, tag="xsq")
                    nc.vector.tensor_mul(
                        xsq[:].rearrange("p a b -> p (a b)"),
                        xb16[:].rearrange("p a b -> p (a b)"),
                        xb16[:].rearrange("p a b -> p (a b)"),
                    )
                    ps_xnb = pp.tile([128, C], F32, tag="ps_xnb", bufs=1)
                    for kc in range(2):
                        for no, nn_ in _n_slices(C):
                            nc.tensor.matmul(
                                ps_xnb[:, no : no + nn_],
                                ones[:, :],
                                xsq[:, kc, no : no + nn_],
                                start=(kc == 0),
                                stop=(kc == 1),
                            )
                    xnb = tp.tile([128, C], F16, tag="xnb")
                    nc.scalar.copy(xnb[:], ps_xnb[:])

                    h1 = tp.tile([128, 3, C], F8, tag="h1")
                    for tcn in range(3):
                        ps = dot1s[tcn]
                        sq = tp.tile([128, C], F16, tag="sq", bufs=3)
                        nc.scalar.activation(
                            sq[:], ps[:], AF.Square,
                            bias=sqbt[:, tcn : tcn + 1], scale=-16.0,
                        )
                        rec = tp.tile([128, C], F32, tag="rec", bufs=3)
                        nc.vector._custom_dve(
                            DEN_RECIP_ANT, out=rec[:], in0=ps[:], in1=xnb[:],
                            s0=RC0, s1=wpt[:, tcn : tcn + 1], imm2=RC1,
                        )
                        eng = nc.gpsimd if tcn < 2 else nc.vector
                        eng.tensor_mul(h1[:, tcn, :], sq[:], rec[:])
                    h1s[b] = h1

                def x2_stage(b):
                    xb16 = xb16s[b]
                    h1 = h1s[b]
                    r0 = b * P
                    for mc in range(6):
                        ps_x2 = pp.tile([128, P], F32, tag="ps_x2", bufs=2)
                        nc.tensor.matmul(
                            ps_x2[:],
                            h1[:, 0:2, mc * 128 : (mc + 1) * 128],
                            w2s8T[:, 0:2, :],
                            start=True,
                            stop=False,
                            perf_mode=DR,
                        )
                        nc.tensor.matmul(
                            ps_x2[:],
                            h1[:, 2, mc * 128 : (mc + 1) * 128],
                            w2s8T[:, 2, :],
                            start=False,
                            stop=False,
                        )
                        # shortcut 1024*x.T via sliced identity blocks
                        nc.tensor.matmul(
                            ps_x2[:, 0:128],
                            xb16[0:128, 0, mc * 128 : (mc + 1) * 128],
                            i196[0:128, 0, 0:128],
                            start=False,
                            stop=False,
                        )
                        nc.tensor.matmul(
                            ps_x2[:, 128:196],
                            xb16[0:68, 1, mc * 128 : (mc + 1) * 128],
                            i196[0:68, 1, 128:196],
                            start=False,
                            stop=False,
                        )
                        nc.tensor.matmul(
                            ps_x2[:],
                            ones[0:1, :],
                            b2r[0:1, :],
                            start=False,
                            stop=True,
                        )
                        nc.scalar.activation(
                            x2T[:, mc, r0 : r0 + P], ps_x2[:], AF.Copy,
                            scale=1.0 / 1024.0,
                        )
                        nc.scalar.activation(
                            x2T8[:, mc, r0 : r0 + P], ps_x2[:], AF.Copy,
                            scale=1.0 / 1024.0,
                        )

                for b in range(BL + 1):
                    if b < BL:
                        yat_stage(b)
                    if b > 0:
                        x2_stage(b - 1)

            # ================= Channel-stage row norms =================
            with (
                tc.tile_pool(name="xn_sbuf", bufs=1) as xp,
                tc.tile_pool(name="xn_psum", bufs=1, space="PSUM") as xpp,
            ):
                ps_xn2 = xpp.tile([128, ROWSP], F32)
                for kc in range(6):
                    x2sq = xp.tile([128, ROWSP], F16, tag="x2sq", bufs=2)
                    nc.vector.tensor_mul(x2sq[:], x2T[:, kc, :], x2T[:, kc, :])
                    for no, nn_ in _n_slices(ROWSP):
                        nc.tensor.matmul(
                            ps_xn2[:, no : no + nn_],
                            ones[:, :],
                            x2sq[:, no : no + nn_],
                            start=(kc == 0),
                            stop=(kc == 5),
                        )
                nc.scalar.copy(xn2b[:], ps_xn2[:])

            # ================= Channel stage =================
            with (
                tc.tile_pool(name="ch_sbuf", bufs=2) as chp,
                tc.tile_pool(name="ch_psum", bufs=1, space="PSUM") as cpp,
            ):
                for blk in range(4):
                    r0 = blk * RB
                    rn_out = min(RB, ROWS - r0)   # valid rows to DMA out
                    poT = cpp.tile(
                        [128, 6, RB], F32, tag="poT", bufs=1, name="poT"
                    )
                    h2ps = [None] * 12

                    def pair_stage(mcp):
                        h2p = chp.tile([128, 2, RB], F8, tag="h2p", bufs=3)
                        for j in range(2):
                            mc = 2 * mcp + j
                            ps_d2 = cpp.tile([128, RB], F32, tag="ps_d2", bufs=3)
                            for k in range(3):
                                nc.tensor.matmul(
                                    ps_d2[:],
                                    cw8[:, 2 * k : 2 * k + 2,
                                        mc * 128 : (mc + 1) * 128],
                                    x2T8[:, 2 * k : 2 * k + 2, r0 : r0 + RB],
                                    start=(k == 0),
                                    stop=(k == 2),
                                    perf_mode=DR,
                                )
                            rec = chp.tile([128, RB], F32, tag="rec2", bufs=4)
                            nc.vector._custom_dve(
                                DEN_RECIP_ANT, out=rec[:], in0=ps_d2[:],
                                in1=xn2b[:, r0 : r0 + RB],
                                s0=RC0, s1=wpc[:, mc : mc + 1], imm2=RC1,
                            )
                            if mc % 3 == 0:
                                nc.vector._custom_dve(
                                    SQ_MUL_ANT, out=h2p[:, j, :], in0=ps_d2[:],
                                    in1=rec[:],
                                    s0=-16.0, s1=sqbc[:, mc : mc + 1],
                                )
                            else:
                                sq = chp.tile([128, RB], F16, tag="sq2", bufs=4)
                                nc.scalar.activation(
                                    sq[:], ps_d2[:], AF.Square,
                                    bias=sqbc[:, mc : mc + 1], scale=-16.0,
                                )
                                nc.gpsimd.tensor_mul(h2p[:, j, :], sq[:], rec[:])
                        h2ps[mcp] = h2p

                    def po_stage(mcp):
                        h2p = h2ps[mcp]
                        for cc in range(6):
                            nc.tensor.matmul(
                                poT[:, cc, :],
                                w4T8[:, 2 * mcp : 2 * mcp + 2,
                                     cc * 128 : (cc + 1) * 128],
                                h2p[:, :, :],
                                start=(mcp == 0),
                                stop=(mcp == 11),
                                perf_mode=DR,
                            )

                    for mcp in range(13):
                        if mcp < 12:
                            pair_stage(mcp)
                        if mcp > 0:
                            po_stage(mcp - 1)

                    for cc in range(6):
                        osb = chp.tile([128, RB], F16, tag="osb", bufs=3)
                        nc.vector.affine_then_add(
                            osb[:], poT[:, cc, :], x2T[:, cc, r0 : r0 + RB],
                            scale=1.0 / 1024.0, bias=b4c[:, cc : cc + 1],
                        )
                        nc.sync.dma_start(
                            out_dram[:, cc, r0 : r0 + rn_out], osb[:, 0:rn_out]
                        )

    nc.compile()
    return nc


# ---------------------------------------------------------------------------
# Host-side packing
# ---------------------------------------------------------------------------

def _pack_kpn8(w, n_chunks, scale):
    """(K, N) fp32 -> fp8 (128, n_chunks, N), zero K-padding, pre-scaled."""
    k, n = w.shape
    out = np.zeros((n_chunks * 128, n), np.float32)
    out[:k] = w * scale
    return np.ascontiguousarray(
        out.reshape(n_chunks, 128, n).transpose(1, 0, 2)
    ).astype(NP_F8)


def _pack_col(v, n_chunks):
    """(K,) fp32 -> (128, n_chunks) fp32 column chunks, zero-padded."""
    out = np.zeros((n_chunks * 128,), np.float32)
    out[: v.shape[0]] = v
    return np.ascontiguousarray(out.reshape(n_chunks, 128).T)


_PROGRAM = None


def _get_program():
    global _PROGRAM
    if _PROGRAM is None:
        _PROGRAM = build_program()
    return _PROGRAM


def kernel(x, tw, tb, t_alpha, w2, b2, cw, cb, c_alpha, w4, b4, _trace=False):
    x = np.asarray(x, np.float32)
    tw = np.asarray(tw, np.float32)
    tb = np.asarray(tb, np.float32)
    w2 = np.asarray(w2, np.float32)
    b2 = np.asarray(b2, np.float32)
    cw = np.asarray(cw, np.float32)
    cb = np.asarray(cb, np.float32)
    w4 = np.asarray(w4, np.float32)
    b4 = np.asarray(b4, np.float32)

    scale_t = np.float32(np.sqrt(np.float32(T / np.log(T + 1.0)))) ** np.asarray(
        t_alpha, np.float32
    )[0]
    scale_c = np.float32(np.sqrt(np.float32(M3 / np.log(M3 + 1.0)))) ** np.asarray(
        c_alpha, np.float32
    )[0]

    shared = {
        "twT8": _pack_kpn8(tw.T, 2, -2.0),              # (196,384)
        "w2s8T": _pack_kpn8(w2.T, 3, scale_t),          # (384,196)
        "b2r": (1024.0 * b2).astype(np.float16).reshape(1, P),
        "cw8": _pack_kpn8(cw.T, 6, -2.0),               # (768,3072)
        "w4T8": _pack_kpn8(w4.T, 24, scale_c),          # (3072,768)
        "sqbt": _pack_col(32.0 * tb, 3),
        "wpt": _pack_col((tw ** 2).sum(1) + EPS + 2.0 * tb, 3),
        "sqbc": _pack_col(32.0 * cb, 24),
        "wpc": _pack_col((cw ** 2).sum(1) + EPS + 2.0 * cb, 24),
        "b4c": _pack_col(b4, 6),
    }
    # identity blocks: (128, 2, 196) fp16, chunk kc holds 1024*I rows kc*128..
    i196 = np.zeros((2 * 128, P), np.float32)
    i196[:P] = 1024.0 * np.eye(P, dtype=np.float32)
    shared["i196"] = np.ascontiguousarray(
        i196.reshape(2, 128, P).transpose(1, 0, 2)
    ).astype(np.float16)

    # x: (B, P, C) -> per-core (BL, 128, 2, C), zero row padding
    xp = np.zeros((NCORES, BL, 256, C), np.float32)
    xp[:, :, :P] = x.reshape(NCORES, BL, P, C)
    xp = np.ascontiguousarray(
        xp.reshape(NCORES, BL, 2, 128, C).transpose(0, 1, 3, 2, 4)
    )
    x16 = xp.astype(np.float16)
    x8 = xp.astype(NP_F8)
    in_maps = [dict(shared, x16=x16[c], x8=x8[c]) for c in range(NCORES)]

    nc = _get_program()
    kwargs = {}
    if _trace:
        import shutil, os

        shutil.rmtree("/tmp/bass_ntff", ignore_errors=True)
        os.makedirs("/tmp/bass_ntff", exist_ok=True)
        kwargs["tmpdir"] = "/tmp/bass_ntff"
    res = bass_utils.run_bass_kernel_spmd(
        nc, in_maps, core_ids=list(range(NCORES)), trace=_trace, **kwargs
    )
    # outT: (128, 6, ROWS) fp16, out[cc*128+p, r] -> (ROWS, 768)
    outs = []
    for c in range(NCORES):
        oT = np.asarray(res.results[c]["outT"], np.float32)
        outs.append(oT.transpose(2, 1, 0).reshape(ROWS, C))
    out = np.concatenate(outs, axis=0).reshape(B, P, C)
    if _trace:
        kernel.last_results = res
    return out
